# revision 11
# baseline (speedup 1.0000x reference)
"""Causal self-attention (B=4, T=2048, C=1024, H=16, D=64) on 8 trn2 cores.

Sharding: core c handles batch b = c//2 and head-group hg = c%2 (8 heads).
qkv column-parallel, attention head-parallel, out_proj row-parallel with
per-pair partials; host sums the 8 partials per batch + bias.

All matmul operands are bf16 (inputs cast on host). Per-core program,
software-pipelined over head PAIRS (2 heads = feature partitions 0-63 /
64-127):
  per pair p, per token-chunk tcn (= q-chunk qc):
    - q,k projected feature-major [feat, tok] (DVE evac fuses the bias add)
    - v projected token-major [tok, feat] (so attn@v emits ctx token-major)
    - attention qc: per k-tile: scoresT = k.T@q into psum, exp on ACT
      (scale=1/8) -> e bf16, diag triangle mask on DVE, then per q-subtile
      ctx[q,d] += e.T@v (N=64) and denom[q] += e.T@ones (N=1) into psum;
      normalization is fused into the ctx psum evacuation (tensor_scalar
      by the reciprocal of the per-partition denominator)
    - ctx transposed via PE into bf16 psum; evac fuses the v-bias add
    - out-proj per (tok-tile, col-half): one N=512 matmul; evac to bf16,
      DMA per-pair partial to DRAM
  A filler queue interleaves qkv(p+1)/transpose(p)/out-proj(p) matmuls into
  the attention stretches (ACT-bound) to keep PE saturated.
"""

import os
import sys
from collections import deque

for _p in ("/opt/trn_rl_repo", "/root/.axon_site/_ro/trn_rl_repo"):
    if os.path.isdir(_p) and _p not in sys.path:
        sys.path.insert(0, _p)

import numpy as np

B, T, C = 4, 2048, 1024
H, D = 16, 64
NCORES = 8
HPC = 8          # heads per core
FQ = HPC * D     # 512 per-core q (=k=v) feature count
NPAIR = 4        # head pairs per core
FILL_NS = 320.0  # PE filler budget per attention k-tile step

_CACHE = {}


class _Filler:
    """FIFO of (tag, closure, pe_ns) emit units, pulled lazily."""

    def __init__(self):
        self.q = deque()

    def add(self, tag, units):
        for fn, ns in units:
            self.q.append((tag, fn, ns))

    def emit(self, budget_ns):
        spent = 0.0
        while self.q and spent < budget_ns:
            _, fn, ns = self.q.popleft()
            fn()
            spent += ns

    def drain(self, tag):
        """Emit everything up to and including the last unit tagged `tag`."""
        if not any(t == tag for t, _, _ in self.q):
            return
        while self.q:
            t, fn, _ = self.q.popleft()
            fn()
            if t == tag and not any(x == tag for x, _, _ in self.q):
                break

    def drain_all(self):
        while self.q:
            _, fn, _ = self.q.popleft()
            fn()


def _build_program():
    import concourse.bacc as bacc
    import concourse.tile as tile
    import concourse.mybir as mybir
    from contextlib import ExitStack

    f32 = mybir.dt.float32
    bf16 = mybir.dt.bfloat16
    AF = mybir.ActivationFunctionType

    nc = bacc.Bacc("TRN2", target_bir_lowering=False, debug=False)

    x_t = nc.dram_tensor("x_t", [C, T], bf16, kind="ExternalInput").ap()
    w_s = nc.dram_tensor("w_s", [C, 3 * FQ], bf16, kind="ExternalInput").ap()
    b_s = nc.dram_tensor("b_s", [3 * FQ], f32, kind="ExternalInput").ap()
    w_o = nc.dram_tensor("w_o", [FQ, C], bf16, kind="ExternalInput").ap()
    tri_d = nc.dram_tensor("tri", [128, 128], bf16, kind="ExternalInput").ap()
    idm_d = nc.dram_tensor("idm", [128, 128], bf16, kind="ExternalInput").ap()
    y_d = nc.dram_tensor("y", [NPAIR, T, C], bf16, kind="ExternalOutput").ap()

    MM = 0.4167  # ns per matmul output column (cost bookkeeping only)

    with tile.TileContext(nc) as tc, ExitStack() as ctx:
        # ---- whole-kernel persistents ----
        pp = ctx.enter_context(tc.tile_pool(name="persist", bufs=1))
        ident = pp.tile([128, 128], bf16, tag="ident", name="ident")
        tri_sb = pp.tile([128, 128], bf16, tag="tri", name="tri_sb")
        b_sb = pp.tile([128, 12], f32, tag="bias", name="b_sb")
        ones_col = pp.tile([128, 1], bf16, tag="ones", name="ones_col")
        nc.sync.dma_start(out=ident, in_=idm_d)
        nc.sync.dma_start(out=tri_sb, in_=tri_d)
        nc.sync.dma_start(out=b_sb, in_=b_s.rearrange("(f p) -> p f", p=128))
        nc.vector.memset(ones_col, 1.0)

        # weights: per (pair, proj) one [128, 8*128] tile; col block ks holds
        # w rows ks*128..(ks+1)*128 for this proj's 128 features
        wqt, wkt, wvt = [], [], []
        for p in range(NPAIR):
            for lst, ft, eng in ((wqt, p, nc.scalar), (wkt, 4 + p, nc.scalar),
                                 (wvt, 8 + p, nc.gpsimd)):
                t = pp.tile([128, 1024], bf16, tag=f"w{ft}", name=f"w_sb{ft}")
                eng.dma_start(
                    out=t.rearrange("p (ks f) -> p ks f", f=128),
                    in_=w_s[:, ft * 128:(ft + 1) * 128].rearrange(
                        "(ks p) f -> p ks f", p=128))
                lst.append(t)
        w_o_sb = []
        for p in range(NPAIR):
            t = pp.tile([128, C], bf16, tag=f"wo{p}", name=f"wo_sb{p}")
            nc.gpsimd.dma_start(out=t, in_=w_o[p * 128:(p + 1) * 128, :])
            w_o_sb.append(t)

        # x: [128, T] per C-chunk, loaded tcn-major so chunk 0 lands first
        x_sb = [pp.tile([128, T], bf16, tag=f"x{ks}", name=f"x_sb{ks}")
                for ks in range(8)]
        for tcn in range(4):
            for ks in range(8):
                eng = nc.sync if ks % 2 == 0 else nc.scalar
                eng.dma_start(
                    out=x_sb[ks][:, tcn * 512:(tcn + 1) * 512],
                    in_=x_t[ks * 128:(ks + 1) * 128,
                            tcn * 512:(tcn + 1) * 512])

        with tc.tile_pool(name="qkp", bufs=2) as qkp, \
             tc.tile_pool(name="v2p", bufs=2) as v2p, \
             tc.tile_pool(name="ctxp", bufs=2) as ctxp, \
             tc.tile_pool(name="ctxTp", bufs=2) as ctxTp, \
             tc.tile_pool(name="ep", bufs=3) as ep, \
             tc.tile_pool(name="rcp", bufs=2) as rcp, \
             tc.tile_pool(name="ysbp", bufs=4) as ysbp, \
             tc.tile_pool(name="scps", bufs=2, space="PSUM") as scps, \
             tc.tile_pool(name="cxps", bufs=1, space="PSUM") as cxps, \
             tc.tile_pool(name="fps", bufs=2, space="PSUM") as fps, \
             tc.tile_pool(name="dnps", bufs=1, space="PSUM") as dnps:

            fill = _Filler()

            # per-pair persistent-ish tiles (rotated via pools)
            qp_t = [None] * NPAIR
            kp_t = [None] * NPAIR
            v2_t = [None] * NPAIR
            ctx_t = [None] * NPAIR
            ctxT_t = [None] * NPAIR
            dn_t = [None] * NPAIR
            rc_t = [None] * NPAIR

            def alloc_pair(p):
                qp_t[p] = qkp.tile([128, T], bf16, tag="qp", name=f"q_{p}")
                kp_t[p] = qkp.tile([128, T], bf16, tag="kp", name=f"k_{p}")
                v2_t[p] = v2p.tile([128, T], bf16, tag="v2", name=f"v_{p}")
                ctx_t[p] = ctxp.tile([128, T], bf16, tag="cx", name=f"cx_{p}")
                ctxT_t[p] = ctxTp.tile([128, T], bf16, tag="cT",
                                       name=f"cT_{p}")
                dn_t[p] = dnps.tile([128, 32], f32, tag="dn", name=f"dn_{p}")
                rc_t[p] = rcp.tile([128, 32], f32, tag="rc", name=f"rc_{p}")

            def qkv_units(p, tcn):
                """Build (closure, pe_ns) units for pair p's qkv @ tcn."""
                c0 = tcn * 512
                units = []

                def qk_proj(wt, dst, bias_col):
                    ps = [None]

                    def mk(ks):
                        def f():
                            if ks == 0:
                                ps[0] = fps.tile([128, 512], f32, tag="fp",
                                                 name="qkv_ps")
                            nc.tensor.matmul(
                                ps[0],
                                lhsT=wt[:, ks * 128:(ks + 1) * 128],
                                rhs=x_sb[ks][:, c0:c0 + 512],
                                start=(ks == 0), stop=(ks == 7))
                        return f

                    for ks in range(8):
                        units.append((mk(ks), 512 * MM))

                    def evac():
                        nc.vector.tensor_scalar_add(
                            dst[:, c0:c0 + 512], ps[0],
                            b_sb[:, bias_col:bias_col + 1])
                    units.append((evac, 0.0))

                def v_proj():
                    # v token-major: out [tok, vfeat] per token tile
                    ps = [None]

                    def mkv(tl, ks):
                        def f():
                            if tl == 0 and ks == 0:
                                ps[0] = fps.tile([128, 512], f32, tag="fp",
                                                 name="v_ps")
                            tt = 4 * tcn + tl
                            nc.tensor.matmul(
                                ps[0][:, tl * 128:(tl + 1) * 128],
                                lhsT=x_sb[ks][:, tt * 128:(tt + 1) * 128],
                                rhs=wvt[p][:, ks * 128:(ks + 1) * 128],
                                start=(ks == 0), stop=(ks == 7))
                        return f

                    for tl in range(4):
                        for ks in range(8):
                            units.append((mkv(tl, ks), 128 * MM))

                    def evacv():
                        nc.vector.tensor_copy(v2_t[p][:, c0:c0 + 512], ps[0])
                    units.append((evacv, 0.0))

                qk_proj(wqt[p], qp_t[p], p)
                qk_proj(wkt[p], kp_t[p], 4 + p)
                v_proj()
                return units

            def transpose_units(p):
                """PE-transpose ctx [q, ft] -> ctxT [ft, q], +v-bias on evac."""
                units = []

                def group(g):
                    tp = [None]

                    def mk(i):
                        def f():
                            if i == 0:
                                tp[0] = fps.tile([128, 512], f32, tag="fp",
                                                 name="tr_ps").bitcast(
                                                     bf16)[:, 0:512]
                            qtg = 4 * g + i
                            nc.tensor.transpose(
                                tp[0][:, i * 128:(i + 1) * 128],
                                ctx_t[p][:, qtg * 128:(qtg + 1) * 128], ident)
                        return f

                    for i in range(4):
                        units.append((mk(i), 128 * MM))

                    def evac():
                        nc.vector.tensor_scalar_add(
                            ctxT_t[p][:, g * 512:(g + 1) * 512], tp[0],
                            b_sb[:, 8 + p:9 + p])
                    units.append((evac, 0.0))

                for g in range(4):
                    group(g)
                return units

            def outproj_units(p):
                units = []
                for tt in range(16):
                    for oc in range(2):
                        def f(tt=tt, oc=oc):
                            yp = fps.tile([128, 512], f32, tag="fp",
                                          name="y_ps")
                            nc.tensor.matmul(
                                yp, lhsT=ctxT_t[p][:, tt * 128:(tt + 1) * 128],
                                rhs=w_o_sb[p][:, oc * 512:(oc + 1) * 512],
                                start=True, stop=True)
                            ysb = ysbp.tile([128, 512], bf16, tag="ysb",
                                            name="y_sb")
                            if (tt + oc) % 2 == 0:
                                nc.vector.tensor_copy(ysb, yp)
                            else:
                                nc.gpsimd.tensor_copy(ysb, yp)
                            nc.sync.dma_start(
                                out=y_d[p, tt * 128:(tt + 1) * 128,
                                        oc * 512:(oc + 1) * 512],
                                in_=ysb)
                        units.append((f, 512 * MM))
                return units

            def attention_qc(p, qc):
                """Emit attention for q-chunk qc of pair p (k-tiles 0..nkt)."""
                nkt = 4 * qc + 4
                qbase = qc * 512
                cx = cxps.tile([128, 512], f32, tag="cx", name="cx_ps")
                e_tiles = [None] * nkt

                def scores_exp(kt):
                    diag = kt >= 4 * qc
                    r = kt - 4 * qc
                    roff = r * 128 if diag else 0
                    scp = scps.tile([128, 1024], f32, tag="sc", name="sc_ps")
                    for side in range(2):
                        poff = side * 64
                        nc.tensor.matmul(
                            scp[:, side * 512 + roff:(side + 1) * 512],
                            lhsT=kp_t[p][poff:poff + 64,
                                         kt * 128:(kt + 1) * 128],
                            rhs=qp_t[p][poff:poff + 64,
                                        qbase + roff:qbase + 512],
                            start=True, stop=True)
                    e = ep.tile([128, 1024], bf16, tag="e", name="e_sb")
                    ev = e.rearrange("p (s q) -> p s q", s=2)
                    sv = scp.rearrange("p (s q) -> p s q", s=2)
                    nc.scalar.activation(ev[:, :, roff:512], sv[:, :, roff:512],
                                         AF.Exp, scale=0.125)
                    if diag:
                        for side in range(2):
                            c0 = side * 512 + r * 128
                            nc.vector.tensor_mul(e[:, c0:c0 + 128],
                                                 e[:, c0:c0 + 128], tri_sb)
                    e_tiles[kt] = e

                def attn_av(kt):
                    diag = kt >= 4 * qc
                    r = kt - 4 * qc
                    e = e_tiles[kt]
                    for qt in range(r if diag else 0, 4):
                        last = kt == 4 * qc + qt
                        for side in range(2):
                            col = qt * 128 + side * 64
                            nc.tensor.matmul(
                                cx[:, col:col + 64],
                                lhsT=e[:, side * 512 + qt * 128:
                                       side * 512 + (qt + 1) * 128],
                                rhs=v2_t[p][:, kt * 128 + side * 64:
                                            kt * 128 + side * 64 + 64],
                                start=(kt == 0), stop=last,
                                skip_group_check=True)
                            nc.tensor.matmul(
                                dn_t[p][:, qc * 8 + qt * 2 + side:
                                        qc * 8 + qt * 2 + side + 1],
                                lhsT=e[:, side * 512 + qt * 128:
                                       side * 512 + (qt + 1) * 128],
                                rhs=ones_col,
                                start=(kt == 0), stop=last,
                                skip_group_check=True)
                    # early per-qt evac once its diagonal k-tile landed
                    if diag:
                        qt = r
                        dcol = qc * 8 + qt * 2
                        nc.vector.reciprocal(rc_t[p][:, dcol:dcol + 2],
                                             dn_t[p][:, dcol:dcol + 2])
                        for side in range(2):
                            col = qt * 128 + side * 64
                            ocol = (qc * 4 + qt) * 128 + side * 64
                            nc.vector.tensor_scalar_mul(
                                ctx_t[p][:, ocol:ocol + 64],
                                cx[:, col:col + 64],
                                rc_t[p][:, dcol + side:dcol + side + 1])

                for kt in range(nkt):
                    scores_exp(kt)
                    if kt > 0:
                        fill.emit(FILL_NS)
                        attn_av(kt - 1)
                fill.emit(FILL_NS)
                attn_av(nkt - 1)

            # ---------------- main schedule ----------------
            alloc_pair(0)
            fill.add(("qkv", 0, 0), qkv_units(0, 0))
            fill.add(("qkv", 0, 1), qkv_units(0, 1))
            fill.add(("qkv", 0, 2), qkv_units(0, 2))
            fill.add(("qkv", 0, 3), qkv_units(0, 3))

            for p in range(NPAIR):
                for qc in range(4):
                    fill.drain(("qkv", p, qc))
                    if qc == 3 and p + 1 < NPAIR:
                        alloc_pair(p + 1)
                        fill.add(("qkv", p + 1, 0), qkv_units(p + 1, 0))
                    attention_qc(p, qc)
                # pair done: queue next pair's remaining qkv, then this
                # pair's transposes + out-proj
                if p + 1 < NPAIR:
                    for tcn in range(1, 4):
                        fill.add(("qkv", p + 1, tcn), qkv_units(p + 1, tcn))
                fill.add(("tr", p), transpose_units(p))
                fill.add(("op", p), outproj_units(p))
            fill.drain_all()

    nc.compile()
    return nc


def _to_bf16(a):
    import ml_dtypes
    return np.asarray(a, dtype=ml_dtypes.bfloat16)


def _host_inputs(x, w_qkv, b_qkv, w_out):
    tri = (np.arange(128)[:, None] <= np.arange(128)[None, :]).astype(
        np.float32)

    in_maps = []
    for core in range(NCORES):
        b, hg = core // 2, core % 2
        cs = slice(hg * FQ, (hg + 1) * FQ)
        w_slice = np.concatenate(
            [w_qkv[:, cs], w_qkv[:, C + hg * FQ: C + (hg + 1) * FQ],
             w_qkv[:, 2 * C + hg * FQ: 2 * C + (hg + 1) * FQ]], axis=1)
        b_slice = np.concatenate(
            [b_qkv[cs], b_qkv[C + hg * FQ: C + (hg + 1) * FQ],
             b_qkv[2 * C + hg * FQ: 2 * C + (hg + 1) * FQ]])
        in_maps.append({
            "x_t": _to_bf16(np.ascontiguousarray(x[b].T)),
            "w_s": _to_bf16(np.ascontiguousarray(w_slice)),
            "b_s": np.ascontiguousarray(b_slice).astype(np.float32),
            "w_o": _to_bf16(
                np.ascontiguousarray(w_out[hg * FQ:(hg + 1) * FQ, :])),
            "tri": _to_bf16(tri),
            "idm": _to_bf16(np.eye(128, dtype=np.float32)),
        })
    return in_maps


def get_program():
    if "nc" not in _CACHE:
        _CACHE["nc"] = _build_program()
    return _CACHE["nc"]


def kernel(x, w_qkv, b_qkv, w_out, b_out):
    from concourse.bass_utils import run_bass_kernel_spmd

    x = np.asarray(x, dtype=np.float32)
    w_qkv = np.asarray(w_qkv, dtype=np.float32)
    b_qkv = np.asarray(b_qkv, dtype=np.float32)
    w_out = np.asarray(w_out, dtype=np.float32)
    b_out = np.asarray(b_out, dtype=np.float32)

    nc = get_program()
    in_maps = _host_inputs(x, w_qkv, b_qkv, w_out)
    res = run_bass_kernel_spmd(nc, in_maps, core_ids=list(range(NCORES)))

    out = np.empty((B, T, C), dtype=np.float32)
    for b in range(B):
        acc = res.results[2 * b]["y"].astype(np.float32).sum(axis=0)
        acc += res.results[2 * b + 1]["y"].astype(np.float32).sum(axis=0)
        out[b] = acc + b_out
    return out


# revision 17
# speedup vs baseline: 1.0855x; 1.0855x over previous
"""Causal self-attention (B=4, T=2048, C=1024, H=16, D=64) on 8 trn2 cores.

Sharding: core c handles batch b = c//2 and head-group hg = c%2 (8 heads).
qkv column-parallel, attention head-parallel, out_proj row-parallel with
per-pair partials; host sums the 8 partials per batch + bias.

All matmul operands are bf16 (inputs cast on host). Per-core program,
software-pipelined over head PAIRS (2 heads = feature partitions 0-63 /
64-127):
  per pair p, per token-chunk tcn (= q-chunk qc):
    - q,k projected feature-major [feat, tok] (DVE evac fuses the bias add)
    - v projected token-major [tok, feat] (so attn@v emits ctx token-major)
    - attention qc: per k-tile: scoresT = k.T@q into psum, exp on ACT
      (scale=1/8) -> e bf16, diag triangle mask on DVE, then per q-subtile
      ctx[q,d] += e.T@v (N=64) and denom[q] += e.T@ones (N=1) into psum;
      normalization is fused into the ctx psum evacuation (tensor_scalar
      by the reciprocal of the per-partition denominator)
    - ctx transposed via PE into bf16 psum; evac fuses the v-bias add
    - out-proj per (tok-tile, col-half): one N=512 matmul; evac to bf16,
      DMA per-pair partial to DRAM
  A filler queue interleaves qkv(p+1)/transpose(p)/out-proj(p) matmuls into
  the attention stretches (ACT-bound) to keep PE saturated.
"""

import os
import sys
from collections import deque

for _p in ("/opt/trn_rl_repo", "/root/.axon_site/_ro/trn_rl_repo"):
    if os.path.isdir(_p) and _p not in sys.path:
        sys.path.insert(0, _p)

import numpy as np

B, T, C = 4, 2048, 1024
H, D = 16, 64
NCORES = 8
HPC = 8          # heads per core
FQ = HPC * D     # 512 per-core q (=k=v) feature count
NPAIR = 4        # head pairs per core
FILL_NS = 500.0  # PE filler budget per attention k-tile step

_CACHE = {}


class _Filler:
    """FIFO of (tag, closure, pe_ns) emit units, pulled lazily."""

    def __init__(self):
        self.q = deque()

    def add(self, tag, units):
        for fn, ns in units:
            self.q.append((tag, fn, ns))

    def emit(self, budget_ns):
        spent = 0.0
        while self.q and spent < budget_ns:
            _, fn, ns = self.q.popleft()
            fn()
            spent += ns

    def drain(self, tag):
        """Emit everything up to and including the last unit tagged `tag`."""
        if not any(t == tag for t, _, _ in self.q):
            return
        while self.q:
            t, fn, _ = self.q.popleft()
            fn()
            if t == tag and not any(x == tag for x, _, _ in self.q):
                break

    def drain_all(self):
        while self.q:
            _, fn, _ = self.q.popleft()
            fn()


def _build_program():
    import concourse.bacc as bacc
    import concourse.tile as tile
    import concourse.mybir as mybir
    from contextlib import ExitStack

    f32 = mybir.dt.float32
    bf16 = mybir.dt.bfloat16
    AF = mybir.ActivationFunctionType

    nc = bacc.Bacc("TRN2", target_bir_lowering=False, debug=False)

    x_t = nc.dram_tensor("x_t", [C, T], bf16, kind="ExternalInput").ap()
    w_s = nc.dram_tensor("w_s", [C, 3 * FQ], bf16, kind="ExternalInput").ap()
    b_s = nc.dram_tensor("b_s", [3 * FQ], f32, kind="ExternalInput").ap()
    w_o = nc.dram_tensor("w_o", [FQ, C], bf16, kind="ExternalInput").ap()
    tri_d = nc.dram_tensor("tri", [128, 128], bf16, kind="ExternalInput").ap()
    idm_d = nc.dram_tensor("idm", [128, 128], bf16, kind="ExternalInput").ap()
    y_d = nc.dram_tensor("y", [NPAIR, T, C], bf16, kind="ExternalOutput").ap()

    MM = 0.4167  # ns per matmul output column (cost bookkeeping only)

    with tile.TileContext(nc) as tc, ExitStack() as ctx:
        # ---- whole-kernel persistents ----
        pp = ctx.enter_context(tc.tile_pool(name="persist", bufs=1))
        ident = pp.tile([128, 128], bf16, tag="ident", name="ident")
        tri_sb = pp.tile([128, 128], bf16, tag="tri", name="tri_sb")
        b_sb = pp.tile([128, 12], f32, tag="bias", name="b_sb")
        ones_col = pp.tile([128, 1], bf16, tag="ones", name="ones_col")
        nc.vector.memset(ones_col, 1.0)

        # x on the HWDGE path (SP+ACT), tcn-major so chunk 0 lands first;
        # everything else on the gpsimd SWDGE path so it never queues ahead
        # of x on HWDGE.
        x_sb = [pp.tile([128, T], bf16, tag=f"x{ks}", name=f"x_sb{ks}")
                for ks in range(8)]
        for tcn in range(4):
            for ks in range(8):
                eng = nc.sync if ks % 2 == 0 else nc.scalar
                eng.dma_start(
                    out=x_sb[ks][:, tcn * 512:(tcn + 1) * 512],
                    in_=x_t[ks * 128:(ks + 1) * 128,
                            tcn * 512:(tcn + 1) * 512])

        nc.gpsimd.dma_start(out=b_sb,
                            in_=b_s.rearrange("(f p) -> p f", p=128))
        nc.gpsimd.dma_start(out=ident, in_=idm_d)
        nc.gpsimd.dma_start(out=tri_sb, in_=tri_d)

        # weights: per (pair, proj) one [128, 8*128] tile; col block ks holds
        # w rows ks*128..(ks+1)*128 for this proj's 128 features
        wqt, wkt, wvt = [], [], []
        for lst, base in ((wqt, 0), (wkt, 4), (wvt, 8)):
            lst.extend(
                pp.tile([128, 1024], bf16, tag=f"w{base + p}",
                        name=f"w_sb{base + p}") for p in range(NPAIR))
        w_o_sb = [pp.tile([128, C], bf16, tag=f"wo{p}", name=f"wo_sb{p}")
                  for p in range(NPAIR)]
        for p in range(NPAIR):
            for lst, ft in ((wqt, p), (wkt, 4 + p), (wvt, 8 + p)):
                nc.gpsimd.dma_start(
                    out=lst[p].rearrange("p (ks f) -> p ks f", f=128),
                    in_=w_s[:, ft * 128:(ft + 1) * 128].rearrange(
                        "(ks p) f -> p ks f", p=128))
            nc.gpsimd.dma_start(out=w_o_sb[p],
                                in_=w_o[p * 128:(p + 1) * 128, :])

        with tc.tile_pool(name="qkp", bufs=2) as qkp, \
             tc.tile_pool(name="v2p", bufs=2) as v2p, \
             tc.tile_pool(name="ctxp", bufs=2) as ctxp, \
             tc.tile_pool(name="ctxTp", bufs=2) as ctxTp, \
             tc.tile_pool(name="ep", bufs=4) as ep, \
             tc.tile_pool(name="rcp", bufs=2) as rcp, \
             tc.tile_pool(name="ysbp", bufs=4) as ysbp, \
             tc.tile_pool(name="scps", bufs=2, space="PSUM") as scps, \
             tc.tile_pool(name="cxps", bufs=1, space="PSUM") as cxps, \
             tc.tile_pool(name="fps", bufs=2, space="PSUM") as fps, \
             tc.tile_pool(name="dnps", bufs=1, space="PSUM") as dnps:

            fill = _Filler()

            # per-pair persistent-ish tiles (rotated via pools)
            qp_t = [None] * NPAIR
            kp_t = [None] * NPAIR
            v2_t = [None] * NPAIR
            ctx_t = [None] * NPAIR
            ctxT_t = [None] * NPAIR
            dn_t = [None] * NPAIR
            rc_t = [None] * NPAIR

            def alloc_pair(p):
                qp_t[p] = qkp.tile([128, T], bf16, tag="qp", name=f"q_{p}")
                kp_t[p] = qkp.tile([128, T], bf16, tag="kp", name=f"k_{p}")
                v2_t[p] = v2p.tile([128, T], bf16, tag="v2", name=f"v_{p}")
                ctx_t[p] = ctxp.tile([128, T], bf16, tag="cx", name=f"cx_{p}")
                ctxT_t[p] = ctxTp.tile([128, T], bf16, tag="cT",
                                       name=f"cT_{p}")
                dn_t[p] = dnps.tile([128, 32], f32, tag="dn", name=f"dn_{p}")
                rc_t[p] = rcp.tile([128, 32], f32, tag="rc", name=f"rc_{p}")

            def qkv_units(p, tcn):
                """Build (closure, pe_ns) units for pair p's qkv @ tcn."""
                c0 = tcn * 512
                units = []

                def qk_proj(wt, dst, bias_col):
                    ps = [None]

                    def mk(ks):
                        def f():
                            if ks == 0:
                                ps[0] = fps.tile([128, 512], f32, tag="fp",
                                                 name="qkv_ps")
                            nc.tensor.matmul(
                                ps[0],
                                lhsT=wt[:, ks * 128:(ks + 1) * 128],
                                rhs=x_sb[ks][:, c0:c0 + 512],
                                start=(ks == 0), stop=(ks == 7))
                        return f

                    for ks in range(8):
                        units.append((mk(ks), 512 * MM))

                    def evac():
                        nc.vector.tensor_scalar_add(
                            dst[:, c0:c0 + 512], ps[0],
                            b_sb[:, bias_col:bias_col + 1])
                    units.append((evac, 0.0))

                def v_proj():
                    # v token-major: out [tok, vfeat] per token tile
                    ps = [None]

                    def mkv(tl, ks):
                        def f():
                            if tl == 0 and ks == 0:
                                ps[0] = fps.tile([128, 512], f32, tag="fp",
                                                 name="v_ps")
                            tt = 4 * tcn + tl
                            nc.tensor.matmul(
                                ps[0][:, tl * 128:(tl + 1) * 128],
                                lhsT=x_sb[ks][:, tt * 128:(tt + 1) * 128],
                                rhs=wvt[p][:, ks * 128:(ks + 1) * 128],
                                start=(ks == 0), stop=(ks == 7))
                        return f

                    for tl in range(4):
                        for ks in range(8):
                            units.append((mkv(tl, ks), 128 * MM))

                    def evacv():
                        nc.vector.tensor_copy(v2_t[p][:, c0:c0 + 512], ps[0])
                    units.append((evacv, 0.0))

                qk_proj(wqt[p], qp_t[p], p)
                qk_proj(wkt[p], kp_t[p], 4 + p)
                v_proj()
                return units

            def transpose_units(p, g):
                """PE-transpose ctx [q, ft] -> ctxT [ft, q] for q-tiles of
                chunk g, +v-bias fused into the evac."""
                units = []
                tp = [None]

                def mk(i):
                    def f():
                        if i == 0:
                            tp[0] = fps.tile([128, 512], f32, tag="fp",
                                             name="tr_ps").bitcast(
                                                 bf16)[:, 0:512]
                        qtg = 4 * g + i
                        nc.tensor.transpose(
                            tp[0][:, i * 128:(i + 1) * 128],
                            ctx_t[p][:, qtg * 128:(qtg + 1) * 128], ident)
                    return f

                for i in range(4):
                    units.append((mk(i), 128 * MM))

                def evac():
                    nc.vector.tensor_scalar_add(
                        ctxT_t[p][:, g * 512:(g + 1) * 512], tp[0],
                        b_sb[:, 8 + p:9 + p])
                units.append((evac, 0.0))
                return units

            def outproj_units(p, g):
                units = []
                for tt in range(4 * g, 4 * g + 4):
                    for oc in range(2):
                        def f(tt=tt, oc=oc):
                            yp = fps.tile([128, 512], f32, tag="fp",
                                          name="y_ps")
                            nc.tensor.matmul(
                                yp, lhsT=ctxT_t[p][:, tt * 128:(tt + 1) * 128],
                                rhs=w_o_sb[p][:, oc * 512:(oc + 1) * 512],
                                start=True, stop=True)
                            ysb = ysbp.tile([128, 512], bf16, tag="ysb",
                                            name="y_sb")
                            if (tt + oc) % 2 == 0:
                                nc.vector.tensor_copy(ysb, yp)
                            else:
                                nc.gpsimd.tensor_copy(ysb, yp)
                            nc.sync.dma_start(
                                out=y_d[p, tt * 128:(tt + 1) * 128,
                                        oc * 512:(oc + 1) * 512],
                                in_=ysb)
                        units.append((f, 512 * MM))
                return units

            def attention_qc(p, qc):
                """Emit attention for q-chunk qc of pair p (k-tiles 0..nkt)."""
                nkt = 4 * qc + 4
                qbase = qc * 512
                cx = cxps.tile([128, 512], f32, tag="cx", name="cx_ps")
                e_tiles = [None] * nkt

                def scores_exp(kt):
                    diag = kt >= 4 * qc
                    r = kt - 4 * qc
                    roff = r * 128 if diag else 0
                    scp = scps.tile([128, 1024], f32, tag="sc", name="sc_ps")
                    for side in range(2):
                        poff = side * 64
                        nc.tensor.matmul(
                            scp[:, side * 512 + roff:(side + 1) * 512],
                            lhsT=kp_t[p][poff:poff + 64,
                                         kt * 128:(kt + 1) * 128],
                            rhs=qp_t[p][poff:poff + 64,
                                        qbase + roff:qbase + 512],
                            start=True, stop=True)
                    e = ep.tile([128, 1024], bf16, tag="e", name="e_sb")
                    ev = e.rearrange("p (s q) -> p s q", s=2)
                    sv = scp.rearrange("p (s q) -> p s q", s=2)
                    nc.scalar.activation(ev[:, :, roff:512], sv[:, :, roff:512],
                                         AF.Exp, scale=0.125)
                    if diag:
                        for side in range(2):
                            c0 = side * 512 + r * 128
                            nc.vector.tensor_mul(e[:, c0:c0 + 128],
                                                 e[:, c0:c0 + 128], tri_sb)
                    e_tiles[kt] = e

                def attn_av(kt):
                    diag = kt >= 4 * qc
                    r = kt - 4 * qc
                    e = e_tiles[kt]
                    for qt in range(r if diag else 0, 4):
                        last = kt == 4 * qc + qt
                        for side in range(2):
                            col = qt * 128 + side * 64
                            nc.tensor.matmul(
                                cx[:, col:col + 64],
                                lhsT=e[:, side * 512 + qt * 128:
                                       side * 512 + (qt + 1) * 128],
                                rhs=v2_t[p][:, kt * 128 + side * 64:
                                            kt * 128 + side * 64 + 64],
                                start=(kt == 0), stop=last,
                                skip_group_check=True)
                            nc.tensor.matmul(
                                dn_t[p][:, qc * 8 + qt * 2 + side:
                                        qc * 8 + qt * 2 + side + 1],
                                lhsT=e[:, side * 512 + qt * 128:
                                       side * 512 + (qt + 1) * 128],
                                rhs=ones_col,
                                start=(kt == 0), stop=last,
                                skip_group_check=True)
                    # early per-qt evac once its diagonal k-tile landed
                    if diag:
                        qt = r
                        dcol = qc * 8 + qt * 2
                        nc.vector.reciprocal(rc_t[p][:, dcol:dcol + 2],
                                             dn_t[p][:, dcol:dcol + 2])
                        for side in range(2):
                            col = qt * 128 + side * 64
                            ocol = (qc * 4 + qt) * 128 + side * 64
                            nc.vector.tensor_scalar_mul(
                                ctx_t[p][:, ocol:ocol + 64],
                                cx[:, col:col + 64],
                                rc_t[p][:, dcol + side:dcol + side + 1])

                for kt in range(nkt):
                    scores_exp(kt)
                    if kt >= 2:
                        fill.emit(FILL_NS)
                        attn_av(kt - 2)
                for kt in (nkt - 2, nkt - 1):
                    fill.emit(FILL_NS)
                    attn_av(kt)

            # ---------------- main schedule ----------------
            alloc_pair(0)
            fill.add(("qkv", 0, 0), qkv_units(0, 0))
            fill.add(("qkv", 0, 1), qkv_units(0, 1))
            fill.add(("qkv", 0, 2), qkv_units(0, 2))
            fill.add(("qkv", 0, 3), qkv_units(0, 3))

            for p in range(NPAIR):
                for qc in range(4):
                    fill.drain(("qkv", p, qc))
                    if qc == 3 and p + 1 < NPAIR:
                        alloc_pair(p + 1)
                        fill.add(("qkv", p + 1, 0), qkv_units(p + 1, 0))
                    attention_qc(p, qc)
                    # ctx q-chunk qc is final: its transposes + out-proj
                    # tiles become filler work immediately
                    fill.add(("tr", p, qc), transpose_units(p, qc))
                    fill.add(("op", p, qc), outproj_units(p, qc))
                if p + 1 < NPAIR:
                    for tcn in range(1, 4):
                        fill.add(("qkv", p + 1, tcn), qkv_units(p + 1, tcn))
            fill.drain_all()

    nc.compile()
    return nc


def _to_bf16(a):
    import ml_dtypes
    return np.asarray(a, dtype=ml_dtypes.bfloat16)


def _host_inputs(x, w_qkv, b_qkv, w_out):
    tri = (np.arange(128)[:, None] <= np.arange(128)[None, :]).astype(
        np.float32)

    in_maps = []
    for core in range(NCORES):
        b, hg = core // 2, core % 2
        cs = slice(hg * FQ, (hg + 1) * FQ)
        w_slice = np.concatenate(
            [w_qkv[:, cs], w_qkv[:, C + hg * FQ: C + (hg + 1) * FQ],
             w_qkv[:, 2 * C + hg * FQ: 2 * C + (hg + 1) * FQ]], axis=1)
        b_slice = np.concatenate(
            [b_qkv[cs], b_qkv[C + hg * FQ: C + (hg + 1) * FQ],
             b_qkv[2 * C + hg * FQ: 2 * C + (hg + 1) * FQ]])
        in_maps.append({
            "x_t": _to_bf16(np.ascontiguousarray(x[b].T)),
            "w_s": _to_bf16(np.ascontiguousarray(w_slice)),
            "b_s": np.ascontiguousarray(b_slice).astype(np.float32),
            "w_o": _to_bf16(
                np.ascontiguousarray(w_out[hg * FQ:(hg + 1) * FQ, :])),
            "tri": _to_bf16(tri),
            "idm": _to_bf16(np.eye(128, dtype=np.float32)),
        })
    return in_maps


def get_program():
    if "nc" not in _CACHE:
        _CACHE["nc"] = _build_program()
    return _CACHE["nc"]


def kernel(x, w_qkv, b_qkv, w_out, b_out):
    from concourse.bass_utils import run_bass_kernel_spmd

    x = np.asarray(x, dtype=np.float32)
    w_qkv = np.asarray(w_qkv, dtype=np.float32)
    b_qkv = np.asarray(b_qkv, dtype=np.float32)
    w_out = np.asarray(w_out, dtype=np.float32)
    b_out = np.asarray(b_out, dtype=np.float32)

    nc = get_program()
    in_maps = _host_inputs(x, w_qkv, b_qkv, w_out)
    res = run_bass_kernel_spmd(nc, in_maps, core_ids=list(range(NCORES)))

    out = np.empty((B, T, C), dtype=np.float32)
    for b in range(B):
        acc = res.results[2 * b]["y"].astype(np.float32).sum(axis=0)
        acc += res.results[2 * b + 1]["y"].astype(np.float32).sum(axis=0)
        out[b] = acc + b_out
    return out
